# revision 16
# baseline (speedup 1.0000x reference)
"""Trainium2 Bass kernel for masked attention softmax (ragged sequences).

Reference (per batch b):
    qp[k]  = sum_q query[b,0,q] * w[k,q]
    att[s] = sum_k qp[k] * keys[b,s,k]
    out[b] = softmax(where(s < seq_len[b], att, -inf))

Strategy (v4 -- hybrid PE matvec + DVE STT, fp16 stream, tuned split):
  - Sort batches by len desc, deal round-robin to the 8 cores; slot
    extent ext_j = max len over the 8 cores at slot j is baked into the
    single compiled program (cache key = ext tuple).
  - Measured engine rates (HW): PE per-slot matvec ~1.2-1.5 ns/key-col
    + ~40ns/matmul; DVE scalar_tensor_tensor ~215 ns per s-position per
    128-batch tile REGARDLESS of dtype (fused accum blocks the 2x DVE
    perf mode; plain fp16 TT hits 136ns but cannot accumulate).  Both
    engines therefore cost ~1.5-1.7 ns per batch-position -> split the
    work: PE takes the SPLIT longest slots, DVE the rest as 128-batch
    tiles.  fp16 inputs halve HBM traffic at zero compute cost.
  - PE per slot j: matmul(out=score[0:ext, col], lhsT=ktT[:, off:],
    rhs=qpT[:, j]) -- keys stationary (exact ragged col count), query
    projection moving; scores land [s-part, batch-col] in PSUM; sum
    over s via a ones-column matmul on the PE itself; host-crafted
    "-3e4 dot" key columns implement masking; never-written score rows
    (zeroed PSUM) contribute exp(0)=1, removed by a host count row;
    reciprocal_approx_fast + PE outer-product broadcast + DVE scale.
  - DVE tiles: per position one STT (keys_aug*1.0)*qp_aug, fp16 in /
    fp32 out+accum; mask folded into keys element 128 (0 / -3e4),
    element 129 pads rows to 4B alignment.  ACT exp with accum_out,
    DVE reciprocal [P,1], ACT scale.
  - Keys chunks for both layouts stream on the sync HWDGE queue in
    consumption-deadline order (a naive interleave starves the PE).
  - fp16 keys/qp quantization: max rel err ~4.5e-3 vs fp64 (gate 2e-2);
    exp stays fp32 (scores up to ~60 overflow fp16).
"""

import sys

import numpy as np

sys.path.insert(0, "/opt/trn_rl_repo")

import concourse.bass as bass
import concourse.tile as tile
from concourse import bacc, mybir
from concourse.bass_utils import run_bass_kernel_spmd


def _install_trace_shims():
    """The agent image lacks ``antenv.axon_hooks``, so trace=True silently
    degrades.  Recreate the module and register the ctypes NTFF hook from
    trn_agent_boot; also make artifact upload failure non-fatal."""
    try:
        import types

        import antenv
        from concourse import bass_utils as _bu

        if "antenv.axon_hooks" not in sys.modules:
            mod = types.ModuleType("antenv.axon_hooks")
            mod._hook = None
            mod.set_axon_ntff_profile_hook = lambda h: setattr(mod, "_hook", h)
            mod.get_axon_ntff_profile_hook = lambda: mod._hook
            sys.modules["antenv.axon_hooks"] = mod
            antenv.axon_hooks = mod
            from trn_agent_boot.trn_boot import _ntff_profile_via_ctypes

            mod.set_axon_ntff_profile_hook(
                _ntff_profile_via_ctypes("/opt/axon/libaxon_pjrt.so")
            )

        _orig_upload = _bu.upload_artifacts

        def _safe_upload(tmpdir):
            try:
                return _orig_upload(tmpdir)
            except Exception:
                return "local://" + str(tmpdir)

        _bu.upload_artifacts = _safe_upload
    except Exception:
        pass


_install_trace_shims()

B, S, KD, QD = 4096, 200, 128, 128
NCORES = 8
P = 128
NSLOTS = B // NCORES          # 512 slots (batches) per core
KAUG = 130                    # keys + mask elem + pad elem (4B-aligned rows)
BIGNEG = -30000.0
SPLIT = 224                   # slots [0, SPLIT) on PE; rest on DVE tiles
QWC = KD + NSLOTS + P         # qw cols (zero pad so partial tiles slice OK)

# consumption-rate model for the DMA feed schedule (ns)
T0_PE, RATE_PE_COL = 8800.0, 1.25
T0_DVE, RATE_DVE_POS = 9500.0, 215.0
DMA_NS_PER_BYTE = 1.0 / 420.0 / 16.0 * 16  # ~420 GB/s aggregate

LAST_RESULTS = None
_nc_cache = {}


def _chunk_sizes(total, ramp, steady):
    sizes, c = [], 0
    for r in ramp:
        if c >= total:
            break
        s = min(r, total - c)
        sizes.append(s)
        c += s
    while c < total:
        s = min(steady, total - c)
        sizes.append(s)
        c += s
    return sizes


def _dve_tiles(exts):
    tiles = []
    s0 = SPLIT
    while s0 < NSLOTS:
        s1 = min(s0 + P, NSLOTS)
        tiles.append((s0, s1, int(exts[s0])))
        s0 = s1
    return tiles


def _build(exts):
    exts = list(exts)
    f32, f16 = mybir.dt.float32, mybir.dt.float16
    Exp = mybir.ActivationFunctionType.Exp

    pe_slots = list(range(SPLIT))
    los = {j: min(exts[j], P) for j in pe_slots}
    his = {j: exts[j] - los[j] for j in pe_slots}
    offs = {}
    c = 0
    for j in pe_slots:
        offs[j] = c
        c += exts[j]
    TOTP = c
    pe_groups = [(0, P), (P, SPLIT)]
    gh = [max(his[j] for j in range(a, b)) for (a, b) in pe_groups]

    tiles = _dve_tiles(exts)
    NT = len(tiles)
    TOTV = sum(E for (_, _, E) in tiles)

    nc = bacc.Bacc("TRN2", target_bir_lowering=False, debug=False)
    kv_d = nc.dram_tensor("kv", [P, TOTV, KAUG], f16, kind="ExternalInput")
    kt_d = nc.dram_tensor("kt", [P, TOTP], f16, kind="ExternalInput")
    qw_d = nc.dram_tensor("qw", [P, QWC], f16, kind="ExternalInput")
    cn_d = nc.dram_tensor("cn", [1, 2 * P], f32, kind="ExternalInput")
    op_d = nc.dram_tensor("op", [2, S, P], f32, kind="ExternalOutput")
    ov_d = nc.dram_tensor("ov", [NT, P, S], f32, kind="ExternalOutput")

    # --- chunk plans with deadline-ordered feed -------------------------
    kt_sizes = _chunk_sizes(TOTP, [3072, 3072], 4096)
    kt_chunks = []  # (j0, j1, c0, c1)
    ji, c = 0, 0
    for tgt in kt_sizes:
        j0, c0 = ji, c
        while ji < SPLIT and c - c0 < tgt:
            c += exts[ji]
            ji += 1
        kt_chunks.append((j0, ji, c0, c))
        if ji >= SPLIT:
            break
    slot_chunk = {}
    for ci, (j0, j1, c0, c1) in enumerate(kt_chunks):
        for j in range(j0, j1):
            slot_chunk[j] = (ci, c0)

    kv_sizes = _chunk_sizes(TOTV, [16, 24, 32, 40], 48)
    kv_chunks = []
    p0 = 0
    for s_ in kv_sizes:
        kv_chunks.append((p0, s_))
        p0 += s_


    with tile.TileContext(nc) as tc:
        with (
            tc.tile_pool(name="ktp", bufs=1) as ktp,
            tc.tile_pool(name="kvp", bufs=1) as kvp,
            tc.tile_pool(name="small", bufs=1) as smallp,
            tc.tile_pool(name="scr", bufs=16) as scrp,
            tc.tile_pool(name="psum", bufs=1, space=bass.MemorySpace.PSUM) as psump,
        ):
            # Early kv chunks (positions needed before ~33us) interleave
            # with kt on the sync HWDGE ring by consumption deadline; the
            # kv tail rides the ACT ring -- it is starved while sync has
            # queued work, but its data is only needed late.
            qw = smallp.tile([P, QWC], f16, tag="qw")
            nc.sync.dma_start(qw[:], qw_d[:])
            cn = smallp.tile([1, 2 * P], f32, tag="cn")
            nc.gpsimd.dma_start(cn[:], cn_d[:])

            kv_sync = [ci for ci, (p0, n_) in enumerate(kv_chunks)
                       if p0 < 112]
            kv_act = [ci for ci in range(len(kv_chunks)) if ci not in kv_sync]
            kv_tiles = [None] * len(kv_chunks)
            kt_tiles = [None] * len(kt_chunks)

            def mk_kv(ci):
                p0, npos = kv_chunks[ci]
                return kvp.tile([P, npos, KAUG], f16, tag=f"kv{ci}",
                                name=f"kv{ci}")

            for ci in kv_act:
                p0, npos = kv_chunks[ci]
                kv_tiles[ci] = t = mk_kv(ci)
                nc.scalar.dma_start(t[:], kv_d[:, p0 : p0 + npos, :])

            def issue_kt(ci):
                j0, j1, c0, c1 = kt_chunks[ci]
                kt_tiles[ci] = t = ktp.tile([P, c1 - c0], f16,
                                            tag=f"kt{ci}", name=f"kt{ci}")
                nc.sync.dma_start(t[:], kt_d[:, c0:c1])

            # sync ring order: PE runway, then the kv bridge, then the rest
            issue_kt(0)
            issue_kt(1)
            for ci in kv_sync:
                p0, npos = kv_chunks[ci]
                kv_tiles[ci] = t = mk_kv(ci)
                nc.sync.dma_start(t[:], kv_d[:, p0 : p0 + npos, :])
            for ci in range(2, len(kt_chunks)):
                issue_kt(ci)

            # --- qp projections: qpt first (gates the PE batch stream).
            # PSUM is bank-granular (8 x 2KB): qpb uses a 2-buf ring and
            # qpt shares a tile with the late-used recb outer product.
            mix_ps = psump.tile([P, 2 * P], f32, tag="mix")
            qpt_ps = mix_ps[:, 0:SPLIT]
            nc.tensor.matmul(
                qpt_ps, qw[:, 0:KD], qw[:, KD : KD + SPLIT],
                start=True, stop=True,
            )
            qpt = smallp.tile([P, SPLIT], f16, tag="qpt")
            nc.scalar.copy(qpt[:], qpt_ps)

            qpb_ps = [psump.tile([P, P], f32, tag="qpb", bufs=2,
                                 name=f"qpb{t}")
                      for t in range(NT)]
            for t, (s0, s1, E) in enumerate(tiles):
                nc.tensor.matmul(
                    qpb_ps[t][:], qw[:, KD + s0 : KD + s0 + P], qw[:, 0:KD],
                    start=True, stop=True,
                )
            qp_aug = [smallp.tile([P, KAUG], f16, tag=f"qpa{t}", name=f"qpa{t}")
                      for t in range(NT)]
            for t in range(NT):
                nc.scalar.copy(qp_aug[t][:, 0:KD], qpb_ps[t][:])
                nc.vector.memset(qp_aug[t][:, KD : KD + 1], 1.0)
                nc.vector.memset(qp_aug[t][:, KD + 1 : KAUG], 0.0)


            ones_col = smallp.tile([P, 1], f32, tag="ones_col")
            nc.vector.memset(ones_col[:], 1.0)
            ones_row = smallp.tile([1, P], f32, tag="ones_row")
            nc.vector.memset(ones_row[:], 1.0)

            sc_lo = [psump.tile([P, P], f32, tag=f"sclo{i}", name=f"sclo{i}")
                     for i in range(2)]
            sc_hi = [psump.tile([P, P], f32, tag=f"schi{i}", name=f"schi{i}")
                     for i in range(2)]
            for t in sc_lo + sc_hi:
                nc.vector.memset(t[:], 0.0)

            ssum_ps = psump.tile([1, 2 * P], f32, tag="ssum")
            recb_ps = mix_ps
            ssc = smallp.tile([1, 2 * P], f32, tag="ssc")
            rec_t = smallp.tile([1, 2 * P], f32, tag="rec")
            em_lo = [smallp.tile([P, P], f32, tag=f"emlo{i}", name=f"emlo{i}")
                     for i in range(2)]
            em_hi = [smallp.tile([P, P], f32, tag=f"emhi{i}", name=f"emhi{i}")
                     for i in range(2)]
            o_lo = [smallp.tile([P, P], f32, tag=f"olo{i}", name=f"olo{i}")
                    for i in range(2)]
            o_hi = [smallp.tile([P, P], f32, tag=f"ohi{i}", name=f"ohi{i}")
                    for i in range(2)]

            att = [smallp.tile([P, E], f32, tag=f"att{t}", name=f"att{t}")
                   for t, (_, _, E) in enumerate(tiles)]
            tile_base = {}
            base = 0
            for t, (_, _, E) in enumerate(tiles):
                tile_base[t] = base
                base += E

            def emit_dve_stts(t):
                base = tile_base[t]
                E = tiles[t][2]
                for ci, (p0, npos) in enumerate(kv_chunks):
                    lo = max(p0, base)
                    hi = min(p0 + npos, base + E)
                    ck = kv_tiles[ci]
                    for p in range(lo, hi):
                        scr = scrp.tile([P, KAUG], f32, tag="scr")
                        nc.vector.scalar_tensor_tensor(
                            scr[:],
                            ck[:, p - p0, :],
                            1.0,
                            qp_aug[t][:],
                            op0=mybir.AluOpType.mult,
                            op1=mybir.AluOpType.mult,
                            accum_out=att[t][:, p - base : p - base + 1],
                        )

            def emit_dve_softmax(t):
                E = tiles[t][2]
                e_t = smallp.tile([P, E], f32, tag=f"e{t}", name=f"e{t}")
                ssumv = smallp.tile([P, 1], f32, tag=f"ssv{t}", name=f"ssv{t}")
                nc.scalar.activation(
                    e_t[:], att[t][:], Exp, bias=0.0, scale=1.0,
                    accum_out=ssumv[:],
                )
                recv = smallp.tile([P, 1], f32, tag=f"rcv{t}", name=f"rcv{t}")
                nc.vector.reciprocal(recv[:], ssumv[:])
                o_t = smallp.tile([P, E], f32, tag=f"ovt{t}", name=f"ovt{t}")
                nc.scalar.mul(o_t[:], e_t[:], recv[:])
                nc.sync.dma_start(ov_d[t, :, 0:E], o_t[:])

            def emit_pe_mms(g, lo_frac=0.0, hi_frac=1.0):
                base, bend = pe_groups[g]
                n = bend - base
                a, b_ = base + int(n * lo_frac), base + int(n * hi_frac)
                for j in range(a, b_):
                    col = j - base
                    ci, c0 = slot_chunk[j]
                    o = offs[j] - c0
                    ck = kt_tiles[ci]
                    nc.tensor.matmul(
                        sc_lo[g][0 : los[j], col : col + 1],
                        ck[:, o : o + los[j]],
                        qpt[:, j : j + 1],
                        start=True, stop=True,
                    )
                    if his[j] > 0:
                        nc.tensor.matmul(
                            sc_hi[g][0 : his[j], col : col + 1],
                            ck[:, o + P : o + P + his[j]],
                            qpt[:, j : j + 1],
                            start=True, stop=True,
                        )

            def emit_pe_softmax_a(g):
                el = em_lo[g]
                gcols = slice(g * P, (g + 1) * P)
                nc.scalar.activation(el[:], sc_lo[g][:], Exp,
                                     bias=0.0, scale=1.0)
                if gh[g] > 0:
                    nc.scalar.activation(em_hi[g][0 : gh[g], :],
                                         sc_hi[g][0 : gh[g], :],
                                         Exp, bias=0.0, scale=1.0)
                nc.tensor.matmul(
                    ssum_ps[0:1, gcols], ones_col[0:P, 0:1], el[:],
                    start=True, stop=(gh[g] == 0),
                )
                if gh[g] > 0:
                    nc.tensor.matmul(
                        ssum_ps[0:1, gcols], ones_col[0 : gh[g], 0:1],
                        em_hi[g][0 : gh[g], :],
                        start=False, stop=True,
                    )

            def emit_pe_rec(g):
                gcols = slice(g * P, (g + 1) * P)
                nc.vector.tensor_sub(ssc[0:1, gcols], ssum_ps[0:1, gcols],
                                     cn[0:1, gcols])
                nc.vector.reciprocal_approx_fast(rec_t[0:1, gcols],
                                                 ssc[0:1, gcols])

            def emit_pe_recb(g):
                gcols = slice(g * P, (g + 1) * P)
                nc.tensor.matmul(
                    recb_ps[:, gcols], ones_row[0:1, 0:P], rec_t[0:1, gcols],
                    start=True, stop=True,
                )

            def emit_pe_scale(g):
                ol = o_lo[g]
                gcols = slice(g * P, (g + 1) * P)
                nc.vector.tensor_mul(ol[:], em_lo[g][:], recb_ps[:, gcols])
                nc.sync.dma_start(op_d[g, 0:P, :], ol[:])
                if gh[g] > 0:
                    nc.vector.tensor_mul(o_hi[g][0 : gh[g], :],
                                         em_hi[g][0 : gh[g], :],
                                         recb_ps[0 : gh[g], gcols])
                    nc.sync.dma_start(op_d[g, P : P + gh[g], :],
                                      o_hi[g][0 : gh[g], :])

            # --- emission schedule --------------------------------------
            # Per-engine FIFOs: an op is emitted only at a point where its
            # upstream deps will already be done, else the whole queue
            # head-of-line blocks behind it.
            emit_pe_mms(0)                   # PE: g0 batch MMs
            emit_dve_stts(0)                 # DVE: t0 STTs
            emit_pe_mms(1, 0.0, 0.15)        # PE: start of g1
            emit_pe_softmax_a(0)             # ACT exp g0; PE ssum-MMs g0
            emit_pe_mms(1, 0.15, 1.0)        # PE: rest of g1
            emit_dve_stts(1)                 # DVE: t1 STTs
            emit_pe_rec(0)                   # DVE: g0 sub+rec (ssum ready)
            emit_dve_softmax(0)              # DVE recv t0 (exp ready); ACT
            if NT > 2:
                emit_dve_stts(2)             # DVE: t2 STTs
            emit_pe_recb(0)                  # PE: g0 broadcast (rec0 ready)
            emit_pe_softmax_a(1)             # ACT exp g1; PE ssum-MMs g1
            emit_pe_scale(0)                 # DVE scales g0; store
            emit_pe_rec(1)                   # DVE
            emit_pe_recb(1)                  # PE
            emit_dve_softmax(1)
            emit_pe_scale(1)
            if NT > 2:
                emit_dve_softmax(2)

    nc.compile()
    return nc


def _prep(query, keys, seq_len, w):
    query = np.asarray(query)
    keys = np.asarray(keys)
    w = np.asarray(w)
    lens = np.asarray(seq_len).reshape(B).astype(np.int64)

    order = np.argsort(-lens, kind="stable")
    exts = np.maximum(1, np.minimum(S, lens[order[0::NCORES]])).astype(int)

    pe_slots = list(range(SPLIT))
    offs = {}
    c = 0
    for j in pe_slots:
        offs[j] = c
        c += int(exts[j])
    TOTP = c
    pe_groups = [(0, P), (P, SPLIT)]
    gh = [max(max(int(exts[j]) - P, 0) for j in range(a, b))
          for (a, b) in pe_groups]

    tiles = _dve_tiles(exts)
    TOTV = sum(E for (_, _, E) in tiles)

    keys16 = np.ascontiguousarray(keys, dtype=np.float16)
    q32 = query[:, 0, :].astype(np.float32)
    qp32 = q32 @ w.astype(np.float32).T
    q16 = np.ascontiguousarray(q32, dtype=np.float16)
    wT16 = np.ascontiguousarray(w.T, dtype=np.float16)

    in_maps = []
    perms = []
    for cidx in range(NCORES):
        idx = order[cidx::NCORES]
        perms.append(idx)
        l_c = lens[idx]

        kv = np.zeros((P, TOTV, KAUG), dtype=np.float16)
        base = 0
        for (s0, s1, E) in tiles:
            n = s1 - s0
            bidx = idx[s0:s1]
            kv[:n, base : base + E, :KD] = keys16[bidx, :E, :]
            svE = np.arange(E)[None, :]
            kv[:n, base : base + E, KD] = np.where(
                svE < l_c[s0:s1][:, None], 0.0, np.float16(BIGNEG)
            )
            if n < P:
                kv[n:, base : base + E, KD] = np.float16(BIGNEG)
            base += E

        kt = np.zeros((P, TOTP), dtype=np.float16)
        for j in pe_slots:
            e = int(exts[j])
            b = idx[j]
            o = offs[j]
            kt[:, o : o + e] = keys16[b, :e, :].T
            ln = int(l_c[j])
            if ln < e:
                qpb = qp32[b]
                mcol = (qpb * (BIGNEG / float(qpb @ qpb))).astype(np.float16)
                kt[:, o + ln : o + e] = mcol[:, None]

        qw = np.zeros((P, QWC), dtype=np.float16)
        qw[:, :KD] = wT16
        qw[:, KD : KD + NSLOTS] = q16[idx].T

        cnv = np.zeros((1, 2 * P), dtype=np.float32)
        for g, (a, b_) in enumerate(pe_groups):
            for j in range(a, b_):
                e = int(exts[j])
                lo_stale = P - min(e, P)
                hi_stale = gh[g] - max(e - P, 0) if gh[g] > 0 else 0
                cnv[0, g * P + (j - a)] = lo_stale + hi_stale

        in_maps.append({"kv": kv, "kt": kt, "qw": qw, "cn": cnv})
    return lens, exts, perms, in_maps


def kernel(query, keys, seq_len, w):
    global LAST_RESULTS
    lens, exts, perms, in_maps = _prep(query, keys, seq_len, w)

    key = tuple(int(e) for e in exts)
    nc = _nc_cache.get(key)
    if nc is None:
        nc = _build(key)
        _nc_cache[key] = nc

    res = run_bass_kernel_spmd(nc, in_maps, core_ids=list(range(NCORES)))
    LAST_RESULTS = res

    tiles = _dve_tiles(exts)
    out = np.zeros((B, S), dtype=np.float32)
    sv = np.arange(S)[None, :]
    for c in range(NCORES):
        dev_p = np.asarray(res.results[c]["op"])   # [2, S, P]
        dev_v = np.asarray(res.results[c]["ov"])   # [NT, P, S]
        idx = perms[c]
        l_c = lens[idx]

        full = np.zeros((NSLOTS, S), dtype=np.float32)
        full[0:P, :] = dev_p[0].T
        full[P:SPLIT, :] = dev_p[1].T[: SPLIT - P]
        for t, (s0, s1, E) in enumerate(tiles):
            full[s0:s1, :] = dev_v[t][: s1 - s0]

        arr = np.where(sv < l_c[:, None], full, 0.0).astype(np.float32)
        arr[l_c == 0] = np.float32(1.0 / S)
        out[idx] = arr
    return out
